# revision 55
# baseline (speedup 1.0000x reference)
"""Trainium2 Bass kernel for nn_CustomLoss (4->64->64->1 MLP logprob loss).

Contract: kernel(**inputs) takes FULL unsharded inputs (states [2097152, 4],
W1 [4,64], b1 [64], W2 [64,64], b2 [64], W3 [64,1], b3 [1]) and returns the
full output: loss [1] = -(sum(net(states)) - N * logsumexp(net(eye(4)))).

Strategy (data-parallel over 8 NeuronCores):
  - Shard states along N: 262144 rows per core; replicate tiny MLP weights.
  - Host pre-transposes each shard into xt[32B+4rho+f, 32m+cc] layout so
    features sit on SBUF partitions and each matmul column carries 2 rows.
  - Per core, per 16384-row superblock (all matmul dtypes float32r: full
    fp32 storage, 1 cycle/column PE rate):
      * mm1 (x16, one per (B,g) pair): lhsT = wk1_g[32B:32B+32,:] is a
        row-pair selector (x) W1 block-diag; rhs = xt[32B:32B+32,:].
        Output z1[64r+j, (m,cc)] in PSUM for rows 8m+2g+r.
      * DVE tensor_scalar: h1 = max(z1 + b1, 0)  (bias+relu fused, psum->sbuf)
      * mm2: lhsT = wk2 = diag(W2, W2); rhs = h1 -> z2 in PSUM
      * ACT activation (in place): relu(z2 + b2) with accum_out writing the
        per-partition free-dim sum into one column of b_mat. Partition
        q = 64r+j keeps hidden unit j separate, so the host applies W3
        afterwards; no third matmul and no on-device transpose needed.
  - Host: logits_sum = sum_q W3[q mod 64] * B_total[q] + N*b3; the 4-row
    denominator MLP and the final combine run in float64 on host.
"""

import numpy as np

N_TOTAL = 2097152
N_CORES = 8
ROWS_PER_CORE = N_TOTAL // N_CORES       # 262144
SUPER = 16                               # superblocks per core
ROWS_PER_SUPER = ROWS_PER_CORE // SUPER  # 16384
L_PER_PART = ROWS_PER_SUPER // 128       # 128 rows per partition per superblock
TILES_PER_SUPER = 16                     # (B, g) pairs
R = 512                                  # matmul free dim / rows*2 per tile

_CACHE = {}


def _build_module(n_super=SUPER):
    import concourse.bacc as bacc
    import concourse.tile as tile
    from concourse import mybir

    f32 = mybir.dt.float32
    f32r = mybir.dt.float32r
    nc = bacc.Bacc()

    x = nc.declare_dram_parameter(
        "xt", [n_super, 128, 4 * L_PER_PART], f32r, isOutput=False
    )
    wk1 = nc.declare_dram_parameter("wk1", [128, 4, 128], f32r, isOutput=False)
    wk2 = nc.declare_dram_parameter("wk2", [128, 128], f32r, isOutput=False)
    b1v = nc.declare_dram_parameter("b1v", [128, 1], f32, isOutput=False)
    b2v = nc.declare_dram_parameter("b2v", [128, 1], f32, isOutput=False)
    # (blk, g) sub-tiles per psum group: FD = grp*R = 1024, which with two
    # double-buffered psum pools fills all 8 banks (the optimal split)
    grp = 2
    n_cols = n_super * (TILES_PER_SUPER // grp)
    outp = nc.declare_dram_parameter("out", [128, n_cols], f32, isOutput=True)

    with tile.TileContext(nc) as tc:
        with (
            tc.tile_pool(name="singles", bufs=1) as singles,
            tc.tile_pool(name="xt", bufs=4) as xt_pool,
            tc.tile_pool(name="h1", bufs=4) as h1_pool,
            tc.tile_pool(name="z1", bufs=2, space="PSUM") as z1_pool,
            tc.tile_pool(name="z2", bufs=2, space="PSUM") as z2_pool,
        ):
            wk1_s = singles.tile([128, 4, 128], f32r)
            nc.scalar.dma_start(out=wk1_s, in_=wk1[:])
            wk2_s = singles.tile([128, 128], f32r)
            nc.scalar.dma_start(out=wk2_s, in_=wk2[:])
            b1_s = singles.tile([128, 1], f32)
            nc.scalar.dma_start(out=b1_s, in_=b1v[:])
            b2_s = singles.tile([128, 1], f32)
            nc.scalar.dma_start(out=b2_s, in_=b2v[:])
            b_mat = singles.tile([128, n_cols], f32)
            # dummy activation up front pulls the ~1.3us ACT table load into
            # the initial DMA shadow instead of stalling the first real relu
            warm = singles.tile([128, 1], f32)
            nc.vector.memset(warm, 0.0)
            nc.scalar.activation(
                out=warm, in_=warm, func=mybir.ActivationFunctionType.Relu
            )

            n_acc = TILES_PER_SUPER // grp
            col = 0
            for sb in range(n_super):
                xt_t = xt_pool.tile([128, 4 * L_PER_PART], f32r)
                for blk in range(4):
                    nc.sync.dma_start(
                        out=xt_t[32 * blk : 32 * blk + 32, :],
                        in_=x[sb, 32 * blk : 32 * blk + 32, :],
                    )

                # grp (blk, g) sub-tiles share one wide psum tile so the
                # elementwise passes amortize their per-op overhead
                pairs = [(blk, g) for blk in range(4) for g in range(4)]
                for pi in range(0, 16, grp):
                    t_idx = col + pi // grp
                    z1 = z1_pool.tile([128, grp * R], f32)
                    for k, (blk, g) in enumerate(pairs[pi : pi + grp]):
                        nc.tensor.matmul(
                            out=z1[:, k * R : (k + 1) * R],
                            lhsT=wk1_s[32 * blk : 32 * blk + 32, g, :],
                            rhs=xt_t[32 * blk : 32 * blk + 32, :],
                            start=True,
                            stop=True,
                            tile_position=(32 * blk, 0),
                        )
                    h1 = h1_pool.tile([128, grp * R], f32r)
                    nc.vector.tensor_scalar(
                        out=h1,
                        in0=z1,
                        scalar1=b1_s,
                        scalar2=0.0,
                        op0=mybir.AluOpType.add,
                        op1=mybir.AluOpType.max,
                    )
                    z2 = z2_pool.tile([128, grp * R], f32)
                    for k in range(grp):
                        nc.tensor.matmul(
                            out=z2[:, k * R : (k + 1) * R],
                            lhsT=wk2_s,
                            rhs=h1[:, k * R : (k + 1) * R],
                            start=True,
                            stop=True,
                        )
                    nc.scalar.activation(
                        out=z2,
                        in_=z2,
                        func=mybir.ActivationFunctionType.Relu,
                        bias=b2_s,
                        accum_out=b_mat[:, t_idx : t_idx + 1],
                    )

                nc.sync.dma_start(
                    out=outp[:, col : col + n_acc],
                    in_=b_mat[:, col : col + n_acc],
                )
                col += n_acc

    nc.compile()
    return nc


def _prep_weights(W1, b1, W2, b2):
    """Host-side constant tensors for the device kernel."""
    wk1 = np.zeros((128, 4, 128), dtype=np.float32)
    for g in range(4):
        for blk in range(4):
            for rr in range(2):
                rho = 2 * g + rr
                for f in range(4):
                    wk1[32 * blk + 4 * rho + f, g, 64 * rr : 64 * rr + 64] = W1[f]
    wk2 = np.zeros((128, 128), dtype=np.float32)
    wk2[:64, :64] = W2
    wk2[64:, 64:] = W2
    b1v = np.concatenate([b1, b1]).astype(np.float32).reshape(128, 1)
    b2v = np.concatenate([b2, b2]).astype(np.float32).reshape(128, 1)
    return wk1, wk2, b1v, b2v


def _prep_states(states):
    """Full [N, 4] f32 -> per-core pre-transposed xt [8, SUPER, 128, 512].

    xt[c, s, 32*B + 4*rho + f, 32*m + cc] =
        states[c*ROWS_PER_CORE + s*16384 + (32*B + cc)*128 + 8*m + rho, f]
    """
    x7 = states.reshape(N_CORES, SUPER, 4, 32, 16, 8, 4)  # [c, s, B, cc, m, rho, f]
    xt = np.ascontiguousarray(x7.transpose(0, 1, 2, 5, 6, 4, 3))  # [c,s,B,rho,f,m,cc]
    return xt.reshape(N_CORES, SUPER, 128, 4 * L_PER_PART)


def _host_net_f64(x, W1, b1, W2, b2, W3, b3):
    h = np.maximum(x.astype(np.float64) @ W1.astype(np.float64) + b1, 0.0)
    h = np.maximum(h @ W2.astype(np.float64) + b2, 0.0)
    return h @ W3.astype(np.float64) + b3


def kernel(states, W1, b1, W2, b2, W3, b3):
    from concourse.bass_utils import run_bass_kernel_spmd

    states = np.ascontiguousarray(np.asarray(states, dtype=np.float32))
    W1 = np.asarray(W1, np.float32)
    b1 = np.asarray(b1, np.float32)
    W2 = np.asarray(W2, np.float32)
    b2 = np.asarray(b2, np.float32)
    W3 = np.asarray(W3, np.float32)
    b3 = np.asarray(b3, np.float32)

    if "nc" not in _CACHE:
        _CACHE["nc"] = _build_module()
    nc = _CACHE["nc"]

    wk1, wk2, b1v, b2v = _prep_weights(W1, b1, W2, b2)
    shards = _prep_states(states)
    in_maps = [
        {"xt": shards[c], "wk1": wk1, "wk2": wk2, "b1v": b1v, "b2v": b2v}
        for c in range(N_CORES)
    ]
    res = run_bass_kernel_spmd(nc, in_maps, core_ids=list(range(N_CORES)))

    # Host epilogue in float64.
    w3v = np.concatenate([W3[:, 0], W3[:, 0]]).astype(np.float64)  # [128]
    logits_sum = 0.0
    for c in range(N_CORES):
        bmat = res.results[c]["out"].astype(np.float64)  # [128, TILES]
        logits_sum += float(w3v @ bmat.sum(axis=1))
    logits_sum += N_TOTAL * float(b3[0])

    denom_logits = _host_net_f64(np.eye(4, dtype=np.float64), W1, b1, W2, b2, W3, b3)
    dmax = denom_logits.max()
    log_denom = float(dmax + np.log(np.exp(denom_logits - dmax).sum()))

    loss = -(logits_sum - N_TOTAL * log_denom)
    return np.array([loss], dtype=np.float32)


# revision 57
# speedup vs baseline: 1.0086x; 1.0086x over previous
"""Trainium2 Bass kernel for nn_CustomLoss (4->64->64->1 MLP logprob loss).

Contract: kernel(**inputs) takes FULL unsharded inputs (states [2097152, 4],
W1 [4,64], b1 [64], W2 [64,64], b2 [64], W3 [64,1], b3 [1]) and returns the
full output: loss [1] = -(sum(net(states)) - N * logsumexp(net(eye(4)))).

Strategy (data-parallel over 8 NeuronCores):
  - Shard states along N: 262144 rows per core; replicate tiny MLP weights.
  - Host pre-transposes each shard into xt[32B+4rho+f, 32m+cc] layout so
    features sit on SBUF partitions and each matmul column carries 2 rows.
  - Per core, per 16384-row superblock (all matmul dtypes float32r: full
    fp32 storage, 1 cycle/column PE rate):
      * mm1 (x16, one per (B,g) pair): lhsT = wk1_g[32B:32B+32,:] is a
        row-pair selector (x) W1 block-diag; rhs = xt[32B:32B+32,:].
        Output z1[64r+j, (m,cc)] in PSUM for rows 8m+2g+r.
      * DVE tensor_scalar: h1 = max(z1 + b1, 0)  (bias+relu fused, psum->sbuf)
      * mm2: lhsT = wk2 = diag(W2, W2); rhs = h1 -> z2 in PSUM
      * ACT activation (in place): relu(z2 + b2) with accum_out writing the
        per-partition free-dim sum into one column of b_mat. Partition
        q = 64r+j keeps hidden unit j separate, so the host applies W3
        afterwards; no third matmul and no on-device transpose needed.
  - Host: logits_sum = sum_q W3[q mod 64] * B_total[q] + N*b3; the 4-row
    denominator MLP and the final combine run in float64 on host.
"""

import numpy as np

N_TOTAL = 2097152
N_CORES = 8
ROWS_PER_CORE = N_TOTAL // N_CORES       # 262144
SUPER = 16                               # superblocks per core
ROWS_PER_SUPER = ROWS_PER_CORE // SUPER  # 16384
L_PER_PART = ROWS_PER_SUPER // 128       # 128 rows per partition per superblock
TILES_PER_SUPER = 16                     # (B, g) pairs
R = 512                                  # matmul free dim / rows*2 per tile

_CACHE = {}


def _build_module(n_super=SUPER, zero_b1=False):
    import concourse.bacc as bacc
    import concourse.tile as tile
    from concourse import mybir

    f32 = mybir.dt.float32
    f32r = mybir.dt.float32r
    nc = bacc.Bacc()

    x = nc.declare_dram_parameter(
        "xt", [n_super, 128, 4 * L_PER_PART], f32r, isOutput=False
    )
    wk1 = nc.declare_dram_parameter("wk1", [128, 4, 128], f32r, isOutput=False)
    wk2 = nc.declare_dram_parameter("wk2", [128, 128], f32r, isOutput=False)
    b1v = nc.declare_dram_parameter("b1v", [128, 1], f32, isOutput=False)
    b2v = nc.declare_dram_parameter("b2v", [128, 1], f32, isOutput=False)
    # (blk, g) sub-tiles per psum group: FD = grp*R = 1024, which with two
    # double-buffered psum pools fills all 8 banks (the optimal split)
    grp = 2
    n_cols = n_super * (TILES_PER_SUPER // grp)
    outp = nc.declare_dram_parameter("out", [128, n_cols], f32, isOutput=True)

    with tile.TileContext(nc) as tc:
        with (
            tc.tile_pool(name="singles", bufs=1) as singles,
            tc.tile_pool(name="xt", bufs=4) as xt_pool,
            tc.tile_pool(name="h1", bufs=4) as h1_pool,
            tc.tile_pool(name="z1", bufs=2, space="PSUM") as z1_pool,
            tc.tile_pool(name="z2", bufs=2, space="PSUM") as z2_pool,
        ):
            wk1_s = singles.tile([128, 4, 128], f32r)
            nc.scalar.dma_start(out=wk1_s, in_=wk1[:])
            wk2_s = singles.tile([128, 128], f32r)
            nc.scalar.dma_start(out=wk2_s, in_=wk2[:])
            b1_s = singles.tile([128, 1], f32)
            nc.scalar.dma_start(out=b1_s, in_=b1v[:])
            b2_s = singles.tile([128, 1], f32)
            nc.scalar.dma_start(out=b2_s, in_=b2v[:])
            b_mat = singles.tile([128, n_cols], f32)
            # dummy activation up front pulls the ~1.3us ACT table load into
            # the initial DMA shadow instead of stalling the first real relu
            warm = singles.tile([128, 1], f32)
            nc.vector.memset(warm, 0.0)
            nc.scalar.activation(
                out=warm, in_=warm, func=mybir.ActivationFunctionType.Relu
            )

            n_acc = TILES_PER_SUPER // grp
            col = 0
            for sb in range(n_super):
                xt_t = xt_pool.tile([128, 4 * L_PER_PART], f32r)
                for blk in range(4):
                    nc.sync.dma_start(
                        out=xt_t[32 * blk : 32 * blk + 32, :],
                        in_=x[sb, 32 * blk : 32 * blk + 32, :],
                    )

                # grp (blk, g) sub-tiles share one wide psum tile so the
                # elementwise passes amortize their per-op overhead
                pairs = [(blk, g) for blk in range(4) for g in range(4)]
                for pi in range(0, 16, grp):
                    t_idx = col + pi // grp
                    z1 = z1_pool.tile([128, grp * R], f32)
                    for k, (blk, g) in enumerate(pairs[pi : pi + grp]):
                        nc.tensor.matmul(
                            out=z1[:, k * R : (k + 1) * R],
                            lhsT=wk1_s[32 * blk : 32 * blk + 32, g, :],
                            rhs=xt_t[32 * blk : 32 * blk + 32, :],
                            start=True,
                            stop=True,
                            tile_position=(32 * blk, 0),
                        )
                    h1 = h1_pool.tile([128, grp * R], f32r)
                    if zero_b1:
                        # immediate-scalar variant avoids the TensorScalarPtr
                        # per-op scalar-register load (~11ns/op)
                        nc.vector.tensor_scalar(
                            out=h1,
                            in0=z1,
                            scalar1=0.0,
                            scalar2=None,
                            op0=mybir.AluOpType.max,
                        )
                    else:
                        nc.vector.tensor_scalar(
                            out=h1,
                            in0=z1,
                            scalar1=b1_s,
                            scalar2=0.0,
                            op0=mybir.AluOpType.add,
                            op1=mybir.AluOpType.max,
                        )
                    z2 = z2_pool.tile([128, grp * R], f32)
                    for k in range(grp):
                        nc.tensor.matmul(
                            out=z2[:, k * R : (k + 1) * R],
                            lhsT=wk2_s,
                            rhs=h1[:, k * R : (k + 1) * R],
                            start=True,
                            stop=True,
                        )
                    nc.scalar.activation(
                        out=z2,
                        in_=z2,
                        func=mybir.ActivationFunctionType.Relu,
                        bias=b2_s,
                        accum_out=b_mat[:, t_idx : t_idx + 1],
                    )

                nc.sync.dma_start(
                    out=outp[:, col : col + n_acc],
                    in_=b_mat[:, col : col + n_acc],
                )
                col += n_acc

    nc.compile()
    return nc


def _prep_weights(W1, b1, W2, b2):
    """Host-side constant tensors for the device kernel."""
    wk1 = np.zeros((128, 4, 128), dtype=np.float32)
    for g in range(4):
        for blk in range(4):
            for rr in range(2):
                rho = 2 * g + rr
                for f in range(4):
                    wk1[32 * blk + 4 * rho + f, g, 64 * rr : 64 * rr + 64] = W1[f]
    wk2 = np.zeros((128, 128), dtype=np.float32)
    wk2[:64, :64] = W2
    wk2[64:, 64:] = W2
    b1v = np.concatenate([b1, b1]).astype(np.float32).reshape(128, 1)
    b2v = np.concatenate([b2, b2]).astype(np.float32).reshape(128, 1)
    return wk1, wk2, b1v, b2v


def _prep_states(states):
    """Full [N, 4] f32 -> per-core pre-transposed xt [8, SUPER, 128, 512].

    xt[c, s, 32*B + 4*rho + f, 32*m + cc] =
        states[c*ROWS_PER_CORE + s*16384 + (32*B + cc)*128 + 8*m + rho, f]
    """
    x7 = states.reshape(N_CORES, SUPER, 4, 32, 16, 8, 4)  # [c, s, B, cc, m, rho, f]
    xt = np.ascontiguousarray(x7.transpose(0, 1, 2, 5, 6, 4, 3))  # [c,s,B,rho,f,m,cc]
    return xt.reshape(N_CORES, SUPER, 128, 4 * L_PER_PART)


def _host_net_f64(x, W1, b1, W2, b2, W3, b3):
    h = np.maximum(x.astype(np.float64) @ W1.astype(np.float64) + b1, 0.0)
    h = np.maximum(h @ W2.astype(np.float64) + b2, 0.0)
    return h @ W3.astype(np.float64) + b3


def kernel(states, W1, b1, W2, b2, W3, b3):
    from concourse.bass_utils import run_bass_kernel_spmd

    states = np.ascontiguousarray(np.asarray(states, dtype=np.float32))
    W1 = np.asarray(W1, np.float32)
    b1 = np.asarray(b1, np.float32)
    W2 = np.asarray(W2, np.float32)
    b2 = np.asarray(b2, np.float32)
    W3 = np.asarray(W3, np.float32)
    b3 = np.asarray(b3, np.float32)

    zb1 = bool(np.all(b1 == 0.0))
    key = ("nc", zb1)
    if key not in _CACHE:
        _CACHE[key] = _build_module(zero_b1=zb1)
    nc = _CACHE[key]

    wk1, wk2, b1v, b2v = _prep_weights(W1, b1, W2, b2)
    shards = _prep_states(states)
    in_maps = [
        {"xt": shards[c], "wk1": wk1, "wk2": wk2, "b1v": b1v, "b2v": b2v}
        for c in range(N_CORES)
    ]
    res = run_bass_kernel_spmd(nc, in_maps, core_ids=list(range(N_CORES)))

    # Host epilogue in float64.
    w3v = np.concatenate([W3[:, 0], W3[:, 0]]).astype(np.float64)  # [128]
    logits_sum = 0.0
    for c in range(N_CORES):
        bmat = res.results[c]["out"].astype(np.float64)  # [128, TILES]
        logits_sum += float(w3v @ bmat.sum(axis=1))
    logits_sum += N_TOTAL * float(b3[0])

    denom_logits = _host_net_f64(np.eye(4, dtype=np.float64), W1, b1, W2, b2, W3, b3)
    dmax = denom_logits.max()
    log_denom = float(dmax + np.log(np.exp(denom_logits - dmax).sum()))

    loss = -(logits_sum - N_TOTAL * log_denom)
    return np.array([loss], dtype=np.float32)
